# revision 49
# baseline (speedup 1.0000x reference)
"""Multi-head self-attention (B=4, N=2048, C=768, H=12, D=64) on 8 TRN2 NeuronCores.

Sharding: (batch, head-group) — core c handles batch c//2, heads (c%2)*6..(c%2)*6+5.
Each core computes its 6 heads' attention plus the partial output projection;
the host sums the two partials per batch and adds the bias terms.

Per-core dataflow (all transpose-free):
  inputs (host-prepped, bf16):
    xt  [896, 1152->2048]  x[b].T padded: rows 0..767 = x.T, row 768 = ones, rest 0
    wq  [896, 1152]        cols [q(384) | k(384) | v(384)] for this core's heads;
                           row 768 = [q bias | k bias | 0]
    wp  [384, 768]         proj_w rows for this core's heads
  phase 1: QT,KT [384, 2048] = wq[:, :768].T @ xt   (bias via ones-row)
           V_aug [2048, (6, 96)] = xt.T @ wq[:, 768:]  (+ ones blocks)
  phase 2 per head: S^T tile = KT_h_tile.T @ QT_h -> exp (scale fused) ->
           psum += [v|ones].T @ E^T  (denominator via ones cols) ->
           reciprocal + partition_broadcast -> normalize into OUT^T
  phase 3: partial = OUT^T.T @ wp -> DRAM
Host: out[b] = part[2b] + part[2b+1] + (qkv_b_v @ proj_w + proj_b)
"""

import numpy as np
import ml_dtypes

B, N, C = 4, 2048, 768
H, D = 12, 64
SCALE = D ** -0.5
HL = 6            # heads per core
QK = HL * D       # 384, width of q (= k = v) section per core
KS = 7            # K subtiles (896 = 7*128 rows incl ones/bias row + pad)
P = 128
NT = N            # tokens
NCH = 4           # Nq chunks of 512
SC = 512
MT = N // P       # 16 token tiles / Nk tiles

_cache = {}


def _build():
    import concourse.bass as bass
    import concourse.mybir as mybir
    import concourse.tile as tile
    from concourse import bacc

    f32 = mybir.dt.float32
    bf16 = mybir.dt.bfloat16

    nc = bacc.Bacc(None, target_bir_lowering=False)
    xt_d = nc.declare_dram_parameter("xt", [KS * P, NT], bf16, isOutput=False)
    wq_d = nc.declare_dram_parameter("wq", [KS * P, 3 * QK], bf16, isOutput=False)
    wp_d = nc.declare_dram_parameter("wp", [QK, C], bf16, isOutput=False)
    bias_d = nc.declare_dram_parameter("bias_qk", [P, 2 * QK // P], f32, isOutput=False)
    out_d = nc.declare_dram_parameter("out", [NT, C], f32, isOutput=True)

    xt_r = xt_d.rearrange("(o p) n -> p o n", p=P)
    wq_r = wq_d.rearrange("(o p) n -> p o n", p=P)
    wp_r = wp_d.rearrange("(o p) n -> p o n", p=P)

    with tile.TileContext(nc) as tc:
        with (
            tc.tile_pool(name="persist", bufs=1) as persist,
            tc.tile_pool(name="e_pool", bufs=3) as e_pool,
            tc.tile_pool(name="un_pool", bufs=2) as un_pool,
            tc.tile_pool(name="rec_pool", bufs=4) as rec_pool,
            tc.tile_pool(name="bc_pool", bufs=4) as bc_pool,
            tc.tile_pool(name="stage_pool", bufs=3) as stage_pool,
            tc.tile_pool(name="dr", bufs=4, space="DRAM") as dr_pool,
            tc.tile_pool(name="psS0", bufs=1, space="PSUM") as psS0,
            tc.tile_pool(name="psS1", bufs=1, space="PSUM") as psS1,
            tc.tile_pool(name="psO0", bufs=1, space="PSUM") as psO0,
            tc.tile_pool(name="psO1", bufs=1, space="PSUM") as psO1,
        ):
            xt = persist.tile([P, KS, NT], bf16)
            wq = persist.tile([P, KS, 3 * QK], bf16)
            wp = persist.tile([P, QK // P, C], bf16)
            # per-head padded Q^T/K^T: head h's 64 dims live at partitions
            # (h%2)*64..+64, the other 64 partitions are zero, so mm2 runs as a
            # full K=128 matmul (K=64 / offset lhsT defeats the hidden
            # weight-load: 389ns vs 216ns per matmul). Separate tiles per head
            # (and per outt subtile) keep the dependency tracking fine-grained.
            qt = [persist.tile([P, NT], bf16, name=f"qt{h}") for h in range(HL)]
            kt = [persist.tile([P, NT], bf16, name=f"kt{h}") for h in range(HL)]
            vv = persist.tile([P, MT, HL, P], bf16)     # V_aug per token-tile/head
            outt = [persist.tile([P, NT], bf16, name=f"outt{o}")
                    for o in range(QK // P)]            # normalized out^T

            # input loads: wq_q + xt alternate sync/scalar queues; the later
            # weight groups go on gpsimd's queue, so each consumer's per-queue
            # tick wait covers only the loads it actually needs
            eng = [nc.sync, nc.scalar, nc.gpsimd]
            li = [0]
            for o in range(KS):
                eng[o % 2].dma_start(wq[:, o, 0:QK], wq_r[:, o, 0:QK])
            for j in range(NCH):
                for o in range(KS):
                    eng[(j * KS + o) % 2].dma_start(
                        xt[:, o, j * SC:(j + 1) * SC], xt_r[:, o, j * SC:(j + 1) * SC]
                    )
            for o in range(KS):
                nc.gpsimd.dma_start(wq[:, o, QK:2 * QK], wq_r[:, o, QK:2 * QK])
            for o in range(KS):
                nc.gpsimd.dma_start(wq[:, o, 2 * QK:3 * QK], wq_r[:, o, 2 * QK:3 * QK])
            for o in range(QK // P):
                nc.gpsimd.dma_start(wp[:, o, :], wp_r[:, o, :])

            # V_aug col layout (M=128 so psum writes start at partition 0):
            #   even heads: [v(64) | ones(32) | zeros(32)]
            #   odd  heads: [zeros(32) | ones(32) | v(64)]
            for h in range(HL):
                nc.vector.memset(vv[:, :, h, 32:96] if h % 2 else vv[:, :, h, 64:96], 1.0)
                nc.vector.memset(vv[:, :, h, 0:32] if h % 2 else vv[:, :, h, 96:128], 0.0)

            def qkt_zeros(heads):
                # zero the off-parity partition half of the padded qt/kt tiles
                for h in heads:
                    zo = 0 if h % 2 else 64
                    nc.gpsimd.memset(qt[h][zo:zo + 64, :], 0.0)
                    nc.gpsimd.memset(kt[h][zo:zo + 64, :], 0.0)

            qkt_zeros([0, 1])

            ps_alt = [psS0, psS1, psO0, psO1]
            alt = [0]
            HW = NT // 2  # half of the Nq axis, per pipeline unit

            def ph_pool():
                pool = ps_alt[alt[0] % 4]
                alt[0] += 1
                return pool

            def qkt_mtile(mi, js=range(NCH)):
                # rows mi*128..mi*128+127 of [Q^T; K^T] (mi 0..2 -> Q, 3..5 -> K)
                dst = qt if mi < 3 else kt
                ti = mi % 3
                for j in js:
                    ps = ph_pool().tile([P, HW], f32, tag="ps")
                    for o in range(KS):
                        nc.tensor.matmul(
                            ps[:, :SC],
                            lhsT=wq[:, o, mi * P:(mi + 1) * P],
                            rhs=xt[:, o, j * SC:(j + 1) * SC],
                            start=(o == 0),
                            stop=(o == KS - 1),
                        )
                    sc = slice(j * SC, (j + 1) * SC)
                    nc.vector.tensor_copy(out=dst[2 * ti][0:64, sc], in_=ps[0:64, :SC])
                    nc.vector.tensor_copy(out=dst[2 * ti + 1][64:P, sc], in_=ps[64:P, :SC])

            def v_mtile(ti):
                ps = ph_pool().tile([P, HW], f32, tag="ps")
                for o in range(KS):
                    nc.tensor.matmul(
                        ps[:, :QK],
                        lhsT=xt[:, o, ti * P:(ti + 1) * P],
                        rhs=wq[:, o, 2 * QK:3 * QK],
                        start=(o == 0),
                        stop=(o == KS - 1),
                    )
                psv = ps[:, :QK].rearrange("p (h d) -> p h d", h=HL)
                # even heads -> cols 0:64, odd heads -> cols 64:128
                nc.vector.tensor_copy(out=vv[:, ti, 0:HL:2, 0:64], in_=psv[:, 0:HL:2, :])
                nc.vector.tensor_copy(out=vv[:, ti, 1:HL:2, 64:128], in_=psv[:, 1:HL:2, :])

            def head(h):
                # two independent half-Nq pipelines keep PE fed while exps run
                t, po = h // 2, (h % 2) * 64
                qt_h = qt[h]
                kt_h = kt[h]
                ps_o = [psO0.tile([P, HW], f32, tag="ps", name="ps_o0"),
                        psO1.tile([P, HW], f32, tag="ps", name="ps_o1")]
                spool = [psS0, psS1]
                for m in range(MT):
                    es = []
                    for u in range(2):
                        ps_s = spool[u].tile([P, HW], f32, tag="ps")
                        for jj in range(2):
                            j = u * 2 + jj
                            nc.tensor.matmul(
                                ps_s[:, jj * SC:(jj + 1) * SC],
                                lhsT=kt_h[:, m * P:(m + 1) * P],
                                rhs=qt_h[:, j * SC:(j + 1) * SC],
                                start=True,
                                stop=True,
                            )
                        e = e_pool.tile([P, HW], bf16)
                        nc.scalar.activation(
                            e[:, :], ps_s[:, :], mybir.ActivationFunctionType.Exp,
                            scale=float(SCALE),
                        )
                        es.append(e)
                    for u in range(2):
                        for jj in range(2):
                            nc.tensor.matmul(
                                ps_o[u][:, jj * SC:(jj + 1) * SC],
                                lhsT=vv[:, m, h, :],
                                rhs=es[u][:, jj * SC:(jj + 1) * SC],
                                start=(m == 0),
                                stop=(m == MT - 1),
                            )
                # fast drain of psum -> sbuf, then normalize off the critical path.
                # Per Nq-half so the final head's normalize overlaps with proj.
                dlane = 64 if h % 2 == 0 else 32  # a lane holding the denominator
                un = un_pool.tile([P, NT], f32)
                for u in range(2):
                    cs = slice(u * HW, (u + 1) * HW)
                    nc.vector.tensor_copy(out=un[:, cs], in_=ps_o[u][:, :])
                # the final head's normalize is chunked finer so the proj
                # waves (which consume outt2 column-by-column) unblock early
                nck = 4 if h == HL - 1 else 2
                cw = NT // nck
                for u in range(nck):
                    cs = slice(u * cw, (u + 1) * cw)
                    # reciprocal of the denominator row, spread over 128 lanes:
                    # row -> DRAM -> [128, cw/128] -> reciprocal -> DRAM -> broadcast.
                    # (a one-lane [1, N] reciprocal costs ~N*6 cycles and blocks DVE)
                    dn = dr_pool.tile([1, cw], f32, name="dn", tag="dn")
                    eng[(li[0] + 0) % 3].dma_start(dn[:, :], un[dlane:dlane + 1, cs])
                    dnp = rec_pool.tile([P, cw // P], f32, name="dnp", tag="dnp")
                    eng[(li[0] + 1) % 3].dma_start(dnp[:, :], dn[0].rearrange("(p f) -> p f", p=P))
                    rcp = rec_pool.tile([P, cw // P], f32, name="rcp", tag="rcp")
                    nc.vector.reciprocal(rcp[:, :], dnp[:, :])
                    rd = dr_pool.tile([1, cw], f32, name="rd", tag="rd")
                    eng[(li[0] + 2) % 3].dma_start(rd[0].rearrange("(p f) -> p f", p=P), rcp[:, :])
                    bc = bc_pool.tile([P, cw], f32, name="bc", tag="bc")
                    eng[(li[0] + 0) % 3].dma_start(
                        bc[:, :],
                        bass.AP(tensor=rd.tensor, offset=rd.offset, ap=[[0, P]] + list(rd.ap)),
                    )
                    li[0] += 1
                    nc.vector.tensor_mul(
                        outt[t][po:po + 64, cs], un[po:po + 64, cs], bc[po:po + 64, :]
                    )

            def proj_wave(tis):
                # waves of <=4 tiles (one psum slot each); the o=0/1 matmuls
                # have no dependency on heads 4/5 and fill the gap while the
                # last head's normalize chain completes
                pss = []
                for ti in tis:
                    ps = ph_pool().tile([P, HW], f32, tag="ps", name="ps_proj")
                    pss.append(ps)
                    for w0, wn in [(0, 512), (512, 256)]:
                        for o in (0, 1):
                            nc.tensor.matmul(
                                ps[:, w0:w0 + wn],
                                lhsT=outt[o][:, ti * P:(ti + 1) * P],
                                rhs=wp[:, o, w0:w0 + wn],
                                start=(o == 0),
                                stop=(o == 1),
                            )
                for ti, ps in zip(tis, pss):
                    # separate accumulation group (start=False adds onto the
                    # bank) so these matmuls' dependency on the last head does
                    # not hold back the o=0/1 group above
                    for w0, wn in [(0, 512), (512, 256)]:
                        nc.tensor.matmul(
                            ps[:, w0:w0 + wn],
                            lhsT=outt[2][:, ti * P:(ti + 1) * P],
                            rhs=wp[:, 2, w0:w0 + wn],
                            start=False,
                            stop=True,
                            skip_group_check=True,
                        )
                    stage = stage_pool.tile([P, C], f32)
                    nc.vector.tensor_copy(out=stage[:, :], in_=ps[:, 0:C])
                    nc.sync.dma_start(out_d[ti * P:(ti + 1) * P, :], stage[:, :])

            # emission order: phase-1 tiles interleaved into head-gap slots
            qkt_mtile(0)
            qkt_mtile(3)
            for ti in range(MT):
                v_mtile(ti)
            qkt_zeros([2, 3])
            head(0)
            head(1)
            qkt_mtile(1)
            qkt_mtile(4)
            qkt_zeros([4, 5])
            head(2)
            head(3)
            qkt_mtile(2)
            qkt_mtile(5)
            head(4)
            head(5)
            for w in range(0, MT, 4):
                proj_wave(list(range(w, w + 4)))

    nc.compile()
    return nc


def _prep_inputs(x, qkv_w, qkv_b):
    bf = ml_dtypes.bfloat16
    in_maps = []
    for c in range(8):
        b, hs = c // 2, (c % 2) * HL
        xt = np.zeros((KS * P, NT), dtype=bf)
        xt[0:C, :] = x[b].T.astype(bf)
        xt[C, :] = 1.0
        wq = np.zeros((KS * P, 3 * QK), dtype=bf)
        for s in range(3):  # q, k, v sections
            cols = qkv_w[:, s * C + hs * D: s * C + (hs + HL) * D]
            wq[0:C, s * QK:(s + 1) * QK] = cols.astype(bf)
        wq[C, 0:QK] = qkv_b[hs * D:(hs + HL) * D].astype(bf)
        wq[C, QK:2 * QK] = qkv_b[C + hs * D: C + (hs + HL) * D].astype(bf)
        qk_bias = np.concatenate([
            qkv_b[hs * D:(hs + HL) * D], qkv_b[C + hs * D: C + (hs + HL) * D]
        ]).astype(np.float32)
        in_maps.append({"xt": xt, "wq": wq,
                        "bias_qk": np.ascontiguousarray(qk_bias.reshape(6, P).T)})
    return in_maps


def kernel(x, qkv_w, qkv_b, proj_w, proj_b):
    from concourse.bass_utils import run_bass_kernel_spmd

    x = np.asarray(x, dtype=np.float32)
    qkv_w = np.asarray(qkv_w, dtype=np.float32)
    qkv_b = np.asarray(qkv_b, dtype=np.float32)
    proj_w = np.asarray(proj_w, dtype=np.float32)
    proj_b = np.asarray(proj_b, dtype=np.float32)

    if "nc" not in _cache:
        _cache["nc"] = _build()
    nc = _cache["nc"]

    bf = ml_dtypes.bfloat16
    in_maps = _prep_inputs(x, qkv_w, qkv_b)
    for c in range(8):
        hs = (c % 2) * HL
        in_maps[c]["wp"] = proj_w[hs * D:(hs + HL) * D, :].astype(bf)

    res = run_bass_kernel_spmd(nc, in_maps, core_ids=list(range(8)))
    parts = [res.results[c]["out"].astype(np.float32) for c in range(8)]

    # v-bias contribution (exact, f32) + proj bias, added once per batch
    const_row = qkv_b[2 * C:] @ proj_w + proj_b
    out = np.empty((B, N, C), dtype=np.float32)
    for b in range(B):
        out[b] = parts[2 * b] + parts[2 * b + 1] + const_row
    return out


# revision 50
# speedup vs baseline: 1.0034x; 1.0034x over previous
"""Multi-head self-attention (B=4, N=2048, C=768, H=12, D=64) on 8 TRN2 NeuronCores.

Sharding: (batch, head-group) — core c handles batch c//2, heads (c%2)*6..(c%2)*6+5.
Each core computes its 6 heads' attention plus the partial output projection;
the host sums the two partials per batch and adds the bias terms.

Per-core dataflow (all transpose-free):
  inputs (host-prepped, bf16):
    xt  [896, 1152->2048]  x[b].T padded: rows 0..767 = x.T, row 768 = ones, rest 0
    wq  [896, 1152]        cols [q(384) | k(384) | v(384)] for this core's heads;
                           row 768 = [q bias | k bias | 0]
    wp  [384, 768]         proj_w rows for this core's heads
  phase 1: QT,KT [384, 2048] = wq[:, :768].T @ xt   (bias via ones-row)
           V_aug [2048, (6, 96)] = xt.T @ wq[:, 768:]  (+ ones blocks)
  phase 2 per head: S^T tile = KT_h_tile.T @ QT_h -> exp (scale fused) ->
           psum += [v|ones].T @ E^T  (denominator via ones cols) ->
           reciprocal + partition_broadcast -> normalize into OUT^T
  phase 3: partial = OUT^T.T @ wp -> DRAM
Host: out[b] = part[2b] + part[2b+1] + (qkv_b_v @ proj_w + proj_b)
"""

import numpy as np
import ml_dtypes

B, N, C = 4, 2048, 768
H, D = 12, 64
SCALE = D ** -0.5
HL = 6            # heads per core
QK = HL * D       # 384, width of q (= k = v) section per core
KS = 7            # K subtiles (896 = 7*128 rows incl ones/bias row + pad)
P = 128
NT = N            # tokens
NCH = 4           # Nq chunks of 512
SC = 512
MT = N // P       # 16 token tiles / Nk tiles

_cache = {}


def _build():
    import concourse.bass as bass
    import concourse.mybir as mybir
    import concourse.tile as tile
    from concourse import bacc

    f32 = mybir.dt.float32
    bf16 = mybir.dt.bfloat16

    nc = bacc.Bacc(None, target_bir_lowering=False)
    xt_d = nc.declare_dram_parameter("xt", [KS * P, NT], bf16, isOutput=False)
    wq_d = nc.declare_dram_parameter("wq", [KS * P, 3 * QK], bf16, isOutput=False)
    wp_d = nc.declare_dram_parameter("wp", [QK, C], bf16, isOutput=False)
    bias_d = nc.declare_dram_parameter("bias_qk", [P, 2 * QK // P], f32, isOutput=False)
    out_d = nc.declare_dram_parameter("out", [NT, C], f32, isOutput=True)

    xt_r = xt_d.rearrange("(o p) n -> p o n", p=P)
    wq_r = wq_d.rearrange("(o p) n -> p o n", p=P)
    wp_r = wp_d.rearrange("(o p) n -> p o n", p=P)

    with tile.TileContext(nc) as tc:
        with (
            tc.tile_pool(name="persist", bufs=1) as persist,
            tc.tile_pool(name="e_pool", bufs=4) as e_pool,
            tc.tile_pool(name="un_pool", bufs=2) as un_pool,
            tc.tile_pool(name="rec_pool", bufs=4) as rec_pool,
            tc.tile_pool(name="bc_pool", bufs=4) as bc_pool,
            tc.tile_pool(name="stage_pool", bufs=3) as stage_pool,
            tc.tile_pool(name="dr", bufs=4, space="DRAM") as dr_pool,
            tc.tile_pool(name="psS0", bufs=1, space="PSUM") as psS0,
            tc.tile_pool(name="psS1", bufs=1, space="PSUM") as psS1,
            tc.tile_pool(name="psO0", bufs=1, space="PSUM") as psO0,
            tc.tile_pool(name="psO1", bufs=1, space="PSUM") as psO1,
        ):
            xt = persist.tile([P, KS, NT], bf16)
            wq = persist.tile([P, KS, 3 * QK], bf16)
            wp = persist.tile([P, QK // P, C], bf16)
            # per-head padded Q^T/K^T: head h's 64 dims live at partitions
            # (h%2)*64..+64, the other 64 partitions are zero, so mm2 runs as a
            # full K=128 matmul (K=64 / offset lhsT defeats the hidden
            # weight-load: 389ns vs 216ns per matmul). Separate tiles per head
            # (and per outt subtile) keep the dependency tracking fine-grained.
            qt = [persist.tile([P, NT], bf16, name=f"qt{h}") for h in range(HL)]
            kt = [persist.tile([P, NT], bf16, name=f"kt{h}") for h in range(HL)]
            vv = persist.tile([P, MT, HL, P], bf16)     # V_aug per token-tile/head
            outt = [persist.tile([P, NT], bf16, name=f"outt{o}")
                    for o in range(QK // P)]            # normalized out^T

            # input loads: wq_q + xt alternate sync/scalar queues; the later
            # weight groups go on gpsimd's queue, so each consumer's per-queue
            # tick wait covers only the loads it actually needs
            eng = [nc.sync, nc.scalar, nc.gpsimd]
            li = [0]
            for o in range(KS):
                eng[o % 2].dma_start(wq[:, o, 0:QK], wq_r[:, o, 0:QK])
            for j in range(NCH):
                for o in range(KS):
                    eng[(j * KS + o) % 2].dma_start(
                        xt[:, o, j * SC:(j + 1) * SC], xt_r[:, o, j * SC:(j + 1) * SC]
                    )
            for o in range(KS):
                nc.gpsimd.dma_start(wq[:, o, QK:2 * QK], wq_r[:, o, QK:2 * QK])
            for o in range(KS):
                nc.gpsimd.dma_start(wq[:, o, 2 * QK:3 * QK], wq_r[:, o, 2 * QK:3 * QK])
            for o in range(QK // P):
                nc.gpsimd.dma_start(wp[:, o, :], wp_r[:, o, :])

            # V_aug col layout (M=128 so psum writes start at partition 0):
            #   even heads: [v(64) | ones(32) | zeros(32)]
            #   odd  heads: [zeros(32) | ones(32) | v(64)]
            for h in range(HL):
                nc.vector.memset(vv[:, :, h, 32:96] if h % 2 else vv[:, :, h, 64:96], 1.0)
                nc.vector.memset(vv[:, :, h, 0:32] if h % 2 else vv[:, :, h, 96:128], 0.0)

            def qkt_zeros(heads):
                # zero the off-parity partition half of the padded qt/kt tiles
                for h in heads:
                    zo = 0 if h % 2 else 64
                    nc.gpsimd.memset(qt[h][zo:zo + 64, :], 0.0)
                    nc.gpsimd.memset(kt[h][zo:zo + 64, :], 0.0)

            qkt_zeros([0, 1])

            ps_alt = [psS0, psS1, psO0, psO1]
            alt = [0]
            HW = NT // 2  # half of the Nq axis, per pipeline unit

            def ph_pool():
                pool = ps_alt[alt[0] % 4]
                alt[0] += 1
                return pool

            def qkt_mtile(mi, js=range(NCH)):
                # rows mi*128..mi*128+127 of [Q^T; K^T] (mi 0..2 -> Q, 3..5 -> K)
                dst = qt if mi < 3 else kt
                ti = mi % 3
                for j in js:
                    ps = ph_pool().tile([P, HW], f32, tag="ps")
                    for o in range(KS):
                        nc.tensor.matmul(
                            ps[:, :SC],
                            lhsT=wq[:, o, mi * P:(mi + 1) * P],
                            rhs=xt[:, o, j * SC:(j + 1) * SC],
                            start=(o == 0),
                            stop=(o == KS - 1),
                        )
                    sc = slice(j * SC, (j + 1) * SC)
                    nc.vector.tensor_copy(out=dst[2 * ti][0:64, sc], in_=ps[0:64, :SC])
                    nc.vector.tensor_copy(out=dst[2 * ti + 1][64:P, sc], in_=ps[64:P, :SC])

            def v_mtile(ti):
                ps = ph_pool().tile([P, HW], f32, tag="ps")
                for o in range(KS):
                    nc.tensor.matmul(
                        ps[:, :QK],
                        lhsT=xt[:, o, ti * P:(ti + 1) * P],
                        rhs=wq[:, o, 2 * QK:3 * QK],
                        start=(o == 0),
                        stop=(o == KS - 1),
                    )
                psv = ps[:, :QK].rearrange("p (h d) -> p h d", h=HL)
                # even heads -> cols 0:64, odd heads -> cols 64:128
                nc.vector.tensor_copy(out=vv[:, ti, 0:HL:2, 0:64], in_=psv[:, 0:HL:2, :])
                nc.vector.tensor_copy(out=vv[:, ti, 1:HL:2, 64:128], in_=psv[:, 1:HL:2, :])

            def head(h):
                # two independent half-Nq pipelines keep PE fed while exps run
                t, po = h // 2, (h % 2) * 64
                qt_h = qt[h]
                kt_h = kt[h]
                ps_o = [psO0.tile([P, HW], f32, tag="ps", name="ps_o0"),
                        psO1.tile([P, HW], f32, tag="ps", name="ps_o1")]
                spool = [psS0, psS1]
                for m in range(MT):
                    es = []
                    for u in range(2):
                        ps_s = spool[u].tile([P, HW], f32, tag="ps")
                        for jj in range(2):
                            j = u * 2 + jj
                            nc.tensor.matmul(
                                ps_s[:, jj * SC:(jj + 1) * SC],
                                lhsT=kt_h[:, m * P:(m + 1) * P],
                                rhs=qt_h[:, j * SC:(j + 1) * SC],
                                start=True,
                                stop=True,
                            )
                        e = e_pool.tile([P, HW], bf16)
                        nc.scalar.activation(
                            e[:, :], ps_s[:, :], mybir.ActivationFunctionType.Exp,
                            scale=float(SCALE),
                        )
                        es.append(e)
                    for u in range(2):
                        for jj in range(2):
                            nc.tensor.matmul(
                                ps_o[u][:, jj * SC:(jj + 1) * SC],
                                lhsT=vv[:, m, h, :],
                                rhs=es[u][:, jj * SC:(jj + 1) * SC],
                                start=(m == 0),
                                stop=(m == MT - 1),
                            )
                # fast drain of psum -> sbuf, then normalize off the critical path.
                # Per Nq-half so the final head's normalize overlaps with proj.
                dlane = 64 if h % 2 == 0 else 32  # a lane holding the denominator
                un = un_pool.tile([P, NT], f32)
                for u in range(2):
                    cs = slice(u * HW, (u + 1) * HW)
                    nc.vector.tensor_copy(out=un[:, cs], in_=ps_o[u][:, :])
                # the final head's normalize is chunked finer so the proj
                # waves (which consume outt2 column-by-column) unblock early
                nck = 4 if h == HL - 1 else 2
                cw = NT // nck
                for u in range(nck):
                    cs = slice(u * cw, (u + 1) * cw)
                    # reciprocal of the denominator row, spread over 128 lanes:
                    # row -> DRAM -> [128, cw/128] -> reciprocal -> DRAM -> broadcast.
                    # (a one-lane [1, N] reciprocal costs ~N*6 cycles and blocks DVE)
                    dn = dr_pool.tile([1, cw], f32, name="dn", tag="dn")
                    eng[(li[0] + 0) % 3].dma_start(dn[:, :], un[dlane:dlane + 1, cs])
                    dnp = rec_pool.tile([P, cw // P], f32, name="dnp", tag="dnp")
                    eng[(li[0] + 1) % 3].dma_start(dnp[:, :], dn[0].rearrange("(p f) -> p f", p=P))
                    rcp = rec_pool.tile([P, cw // P], f32, name="rcp", tag="rcp")
                    nc.vector.reciprocal(rcp[:, :], dnp[:, :])
                    rd = dr_pool.tile([1, cw], f32, name="rd", tag="rd")
                    eng[(li[0] + 2) % 3].dma_start(rd[0].rearrange("(p f) -> p f", p=P), rcp[:, :])
                    bc = bc_pool.tile([P, cw], f32, name="bc", tag="bc")
                    eng[(li[0] + 0) % 3].dma_start(
                        bc[:, :],
                        bass.AP(tensor=rd.tensor, offset=rd.offset, ap=[[0, P]] + list(rd.ap)),
                    )
                    li[0] += 1
                    nc.vector.tensor_mul(
                        outt[t][po:po + 64, cs], un[po:po + 64, cs], bc[po:po + 64, :]
                    )

            def proj_wave(tis):
                # waves of <=4 tiles (one psum slot each); the o=0/1 matmuls
                # have no dependency on heads 4/5 and fill the gap while the
                # last head's normalize chain completes
                pss = []
                for ti in tis:
                    ps = ph_pool().tile([P, HW], f32, tag="ps", name="ps_proj")
                    pss.append(ps)
                    for w0, wn in [(0, 512), (512, 256)]:
                        for o in (0, 1):
                            nc.tensor.matmul(
                                ps[:, w0:w0 + wn],
                                lhsT=outt[o][:, ti * P:(ti + 1) * P],
                                rhs=wp[:, o, w0:w0 + wn],
                                start=(o == 0),
                                stop=(o == 1),
                            )
                for ti, ps in zip(tis, pss):
                    # separate accumulation group (start=False adds onto the
                    # bank) so these matmuls' dependency on the last head does
                    # not hold back the o=0/1 group above
                    for w0, wn in [(0, 512), (512, 256)]:
                        nc.tensor.matmul(
                            ps[:, w0:w0 + wn],
                            lhsT=outt[2][:, ti * P:(ti + 1) * P],
                            rhs=wp[:, 2, w0:w0 + wn],
                            start=False,
                            stop=True,
                            skip_group_check=True,
                        )
                    stage = stage_pool.tile([P, C], f32)
                    nc.vector.tensor_copy(out=stage[:, :], in_=ps[:, 0:C])
                    nc.sync.dma_start(out_d[ti * P:(ti + 1) * P, :], stage[:, :])

            # emission order: phase-1 tiles interleaved into head-gap slots
            qkt_mtile(0)
            qkt_mtile(3)
            for ti in range(MT):
                v_mtile(ti)
            qkt_zeros([2, 3])
            head(0)
            head(1)
            qkt_mtile(1)
            qkt_mtile(4)
            qkt_zeros([4, 5])
            head(2)
            head(3)
            qkt_mtile(2)
            qkt_mtile(5)
            head(4)
            head(5)
            for w in range(0, MT, 4):
                proj_wave(list(range(w, w + 4)))

    nc.compile()
    return nc


def _prep_inputs(x, qkv_w, qkv_b):
    bf = ml_dtypes.bfloat16
    in_maps = []
    for c in range(8):
        b, hs = c // 2, (c % 2) * HL
        xt = np.zeros((KS * P, NT), dtype=bf)
        xt[0:C, :] = x[b].T.astype(bf)
        xt[C, :] = 1.0
        wq = np.zeros((KS * P, 3 * QK), dtype=bf)
        for s in range(3):  # q, k, v sections
            cols = qkv_w[:, s * C + hs * D: s * C + (hs + HL) * D]
            wq[0:C, s * QK:(s + 1) * QK] = cols.astype(bf)
        wq[C, 0:QK] = qkv_b[hs * D:(hs + HL) * D].astype(bf)
        wq[C, QK:2 * QK] = qkv_b[C + hs * D: C + (hs + HL) * D].astype(bf)
        qk_bias = np.concatenate([
            qkv_b[hs * D:(hs + HL) * D], qkv_b[C + hs * D: C + (hs + HL) * D]
        ]).astype(np.float32)
        in_maps.append({"xt": xt, "wq": wq,
                        "bias_qk": np.ascontiguousarray(qk_bias.reshape(6, P).T)})
    return in_maps


def kernel(x, qkv_w, qkv_b, proj_w, proj_b):
    from concourse.bass_utils import run_bass_kernel_spmd

    x = np.asarray(x, dtype=np.float32)
    qkv_w = np.asarray(qkv_w, dtype=np.float32)
    qkv_b = np.asarray(qkv_b, dtype=np.float32)
    proj_w = np.asarray(proj_w, dtype=np.float32)
    proj_b = np.asarray(proj_b, dtype=np.float32)

    if "nc" not in _cache:
        _cache["nc"] = _build()
    nc = _cache["nc"]

    bf = ml_dtypes.bfloat16
    in_maps = _prep_inputs(x, qkv_w, qkv_b)
    for c in range(8):
        hs = (c % 2) * HL
        in_maps[c]["wp"] = proj_w[hs * D:(hs + HL) * D, :].astype(bf)

    res = run_bass_kernel_spmd(nc, in_maps, core_ids=list(range(8)))
    parts = [res.results[c]["out"].astype(np.float32) for c in range(8)]

    # v-bias contribution (exact, f32) + proj bias, added once per batch
    const_row = qkv_b[2 * C:] @ proj_w + proj_b
    out = np.empty((B, N, C), dtype=np.float32)
    for b in range(B):
        out[b] = parts[2 * b] + parts[2 * b + 1] + const_row
    return out
